# revision 4
# baseline (speedup 1.0000x reference)
"""AdaptiveAttention (B=2, S=2048, D=1024, H=16) on 8 TRN2 NeuronCores.

Sharding: query-parallel. Core c (c = 0..7) owns batch b = c//4 and query rows
[512*(c%4), 512*(c%4+1)). K/V projections are computed for the core's own 512
key rows and AllGathered (bf16) within each batch group of 4 cores. Each core
then computes all 16 heads of attention for its 512 queries against the full
2048 keys, applies the per-(head, query) sigmoid-gate / softmax-denominator
scale to the context, and runs the full output projection for its rows.
The host concatenates the 8 disjoint [512, 1024] output blocks.

On-chip layout is "feature-major" (transposed): projections produce Q^T/K^T
directly so scores come out keys-on-partitions, which feeds exp (ScalarE, with
the 1/sqrt(dk) folded into the activation scale) and the P@V matmul without
any on-device transposes. Matmuls are bf16 with f32 PSUM accumulation; scores
use 2-head row-packing (K=64) and P@V uses 2-head column-packing (M=64);
softmax denominators come from ones-vector matmuls column-packed 4 heads wide.
"""

import contextlib
import ctypes
import os
import sys
import types

import numpy as np
import ml_dtypes


# ---------------------------------------------------------------------------
# NTFF profiling hook shim (antenv.axon_hooks is absent in this image).
# Only used when BASS_TRACE is set; harmless otherwise.
# ---------------------------------------------------------------------------
def _install_ntff_hook_shim():
    if "antenv.axon_hooks" in sys.modules:
        return
    try:
        lib = ctypes.CDLL("/opt/axon/libaxon_pjrt.so")
    except OSError:
        return
    if not hasattr(lib, "axon_start_nrt_profile"):
        return
    lib.axon_start_nrt_profile.argtypes = [
        ctypes.POINTER(ctypes.c_int64),
        ctypes.c_size_t,
    ]
    lib.axon_start_nrt_profile.restype = ctypes.c_int64
    lib.axon_stop_nrt_profile.argtypes = [ctypes.c_char_p]
    lib.axon_stop_nrt_profile.restype = ctypes.c_int64

    @contextlib.contextmanager
    def _hook(output_dir, device_ids):
        import jax

        jax.devices()
        if device_ids:
            ids = (ctypes.c_int64 * len(device_ids))(*device_ids)
            rc = lib.axon_start_nrt_profile(ids, len(device_ids))
        else:
            rc = lib.axon_start_nrt_profile(None, 0)
        if rc != 0:
            raise RuntimeError(f"axon_start_nrt_profile rc={rc}")
        try:
            yield
        finally:
            n = lib.axon_stop_nrt_profile(str(output_dir).encode())
            if n < 0:
                raise RuntimeError(f"axon_stop_nrt_profile rc={n}")

    mod = types.ModuleType("antenv.axon_hooks")
    _state = {"hook": _hook}
    mod.get_axon_ntff_profile_hook = lambda: _state["hook"]
    mod.set_axon_ntff_profile_hook = lambda h: _state.__setitem__("hook", h)
    sys.modules["antenv.axon_hooks"] = mod
    try:
        import antenv

        antenv.axon_hooks = mod
    except ImportError:
        pass


_install_ntff_hook_shim()

import concourse.bass as bass  # noqa: E402
import concourse.mybir as mybir  # noqa: E402
import concourse.tile as tile  # noqa: E402
from concourse import bacc  # noqa: E402
from concourse.bass_utils import run_bass_kernel_spmd  # noqa: E402

# ---------------------------------------------------------------------------
# Problem constants (hardcoded; kernel.py must be self-contained)
# ---------------------------------------------------------------------------
B, S, D, H = 2, 2048, 1024, 16
DK = D // H                  # 64
N_CORES = 8
R = 4                        # ranks per batch group
SL = S // R                  # 512 local rows per core
P = 128
DT = D // P                  # 8 feature tiles
NKT = S // P                 # 16 key tiles
SCALE = DK ** -0.5

F32 = mybir.dt.float32
BF16 = mybir.dt.bfloat16
AF = mybir.ActivationFunctionType
BF16_NP = ml_dtypes.bfloat16

_CACHE = {}
LAST_EXEC_TIME_NS = None


def _build():
    nc = bacc.Bacc("TRN2", target_bir_lowering=False, debug=False,
                   num_devices=N_CORES)

    # ---- I/O --------------------------------------------------------------
    xqT = nc.dram_tensor("xqT", [D, SL], BF16, kind="ExternalInput")
    xkT = nc.dram_tensor("xkT", [D, SL], BF16, kind="ExternalInput")
    xvT = nc.dram_tensor("xvT", [D, SL], BF16, kind="ExternalInput")
    wq = nc.dram_tensor("wq", [D, D], BF16, kind="ExternalInput")
    wk = nc.dram_tensor("wk", [D, D], BF16, kind="ExternalInput")
    wv = nc.dram_tensor("wv", [D, D], BF16, kind="ExternalInput")
    wo = nc.dram_tensor("wo", [D, D], BF16, kind="ExternalInput")
    wg = nc.dram_tensor("wg", [D, H], BF16, kind="ExternalInput")
    bq = nc.dram_tensor("bq", [P, DT], F32, kind="ExternalInput")
    bk = nc.dram_tensor("bk", [P, DT], F32, kind="ExternalInput")
    bvb = nc.dram_tensor("bvb", [P, D], F32, kind="ExternalInput")
    bob = nc.dram_tensor("bob", [P, D], F32, kind="ExternalInput")
    bg = nc.dram_tensor("bg", [H, 1], F32, kind="ExternalInput")
    out = nc.dram_tensor("out", [SL, D], F32, kind="ExternalOutput")

    with tile.TileContext(nc) as tc:
        with (
            tc.tile_pool(name="cst", bufs=1) as cst,
            tc.tile_pool(name="wpool", bufs=2) as wpool,
            tc.tile_pool(name="xpool", bufs=2) as xpool,
            tc.tile_pool(name="kvpool", bufs=4) as kvpool,
            tc.tile_pool(name="work", bufs=2) as work,
            tc.tile_pool(name="pt_pool", bufs=4) as ptp,
            tc.tile_pool(name="psA", bufs=2, space="PSUM") as psA,
            tc.tile_pool(name="psB", bufs=2, space="PSUM") as psB,
            tc.tile_pool(name="psC", bufs=1, space="PSUM") as psC,
            tc.tile_pool(name="dram", bufs=1, space="DRAM") as dram,
        ):
            # ---- load weights / inputs -----------------------------------
            def load_w(dram_t):  # rotating weight slot [128, 8, 1024]
                t = wpool.tile([P, DT, D], BF16, tag="wmat")
                nc.sync.dma_start(
                    t[:], dram_t.ap().rearrange("(t p) f -> p t f", p=P))
                return t

            def load_x(dram_t):  # rotating activation slot [128, 8, 512]
                t = xpool.tile([P, DT, SL], BF16, tag="xmat")
                nc.sync.dma_start(
                    t[:], dram_t.ap().rearrange("(t p) f -> p t f", p=P))
                return t

            wk_sb = load_w(wk)
            wv_sb = load_w(wv)
            xk_sb = load_x(xkT)
            xv_sb = load_x(xvT)
            bk_sb = cst.tile([P, DT], F32, name="bk_sb")
            nc.sync.dma_start(bk_sb[:], bk[:])
            bv_sb = cst.tile([P, D], F32, name="bv_sb")
            nc.sync.dma_start(bv_sb[:], bvb[:])

            # ---- K^T projection: ktloc [128, 8, 512] = (p, dims-tile, key)
            ktloc = kvpool.tile([P, DT, SL], BF16, tag="kv")
            for mt in range(DT):
                pp = psA.tile([P, 512], F32, tag="pc")
                for kt in range(DT):
                    nc.tensor.matmul(pp[:], wk_sb[:, kt, 128 * mt:128 * mt + 128],
                                     xk_sb[:, kt, :],
                                     start=(kt == 0), stop=(kt == DT - 1))
                nc.vector.tensor_scalar_add(ktloc[:, mt, :], pp[:],
                                            bk_sb[:, mt:mt + 1])

            # ---- V projection: vloc [128, 4, 1024] = (key-in-tile, keytile, dim)
            vloc = kvpool.tile([P, DT, SL], BF16, tag="kv")
            vloc_v = vloc[:].rearrange("p t k -> p (t k)").rearrange(
                "p (a d) -> p a d", a=R)
            for kb in range(R):
                for c2 in range(2):
                    pp = psA.tile([P, 512], F32, tag="pc")
                    for kt in range(DT):
                        nc.tensor.matmul(
                            pp[:], xv_sb[:, kt, 128 * kb:128 * kb + 128],
                            wv_sb[:, kt, 512 * c2:512 * c2 + 512],
                            start=(kt == 0), stop=(kt == DT - 1))
                    nc.vector.tensor_add(vloc_v[:, kb, 512 * c2:512 * c2 + 512],
                                         pp[:], bv_sb[:, 512 * c2:512 * c2 + 512])

            # ---- AllGather K^T and V within each group of 4 cores --------
            in_bounce = dram.tile([2, P, DT, SL], BF16)
            out_bounce = dram.tile([R, 2, P, DT, SL], BF16)
            nc.sync.dma_start(in_bounce[0], ktloc[:])
            nc.sync.dma_start(in_bounce[1], vloc[:])
            nc.gpsimd.collective_compute(
                "AllGather",
                mybir.AluOpType.bypass,
                replica_groups=[[0, 1, 2, 3], [4, 5, 6, 7]],
                ins=[in_bounce.opt()],
                outs=[out_bounce.opt()],
            )
            # gathered views:
            #   K^T: out_bounce[:, 0] = (rank, p, dims-tile, local key)
            #   V:   out_bounce[:, 1] -> (rank, key-in-tile, keytile, dim)
            vg_view = out_bounce[:, 1].rearrange("r p t k -> r p (t k)").rearrange(
                "r p (a d) -> r p a d", a=R)

            # ---- Q^T projection + gate (overlaps the AllGather) ----------
            wq_sb = load_w(wq)
            xq_sb = load_x(xqT)
            wg_sb = cst.tile([P, DT, H], BF16, name="wg_sb")
            nc.sync.dma_start(
                wg_sb[:], wg.ap().rearrange("(t p) h -> p t h", p=P))
            bq_sb = cst.tile([P, DT], F32, name="bq_sb")
            nc.sync.dma_start(bq_sb[:], bq[:])
            bg_sb = cst.tile([H, 1], F32, name="bg_sb")
            nc.sync.dma_start(bg_sb[:], bg[:])

            qt_sb = cst.tile([P, DT, SL], BF16, name="qt_sb")
            for mt in range(DT):
                pp = psA.tile([P, 512], F32, tag="pc")
                for kt in range(DT):
                    nc.tensor.matmul(pp[:], wq_sb[:, kt, 128 * mt:128 * mt + 128],
                                     xq_sb[:, kt, :],
                                     start=(kt == 0), stop=(kt == DT - 1))
                nc.vector.tensor_scalar_add(qt_sb[:, mt, :], pp[:],
                                            bq_sb[:, mt:mt + 1])

            gate_sb = cst.tile([H, SL], F32, name="gate_sb")
            gp = psC.tile([H, 512], F32, tag="sums")
            for kt in range(DT):
                nc.tensor.matmul(gp[:], wg_sb[:, kt, :], xq_sb[:, kt, :],
                                 start=(kt == 0), stop=(kt == DT - 1))
            nc.scalar.activation(gate_sb[:], gp[:], AF.Sigmoid,
                                 bias=bg_sb[:, 0:1])

            ones_sb = cst.tile([P, 1], BF16, name="ones_sb")
            nc.vector.memset(ones_sb[:], 1.0)

            # ---- attention waves (4 heads per wave) ----------------------
            ctxT = cst.tile([P, DT, SL], BF16, name="ctxT")
            for w in range(4):
                # stream this wave's K^T dims-tiles and V dim-window
                ktw = kvpool.tile([P, 2, R, SL], BF16, tag="kv")
                vw = kvpool.tile([P, R, R, 256], BF16, tag="kv")
                for r_ in range(R):
                    nc.sync.dma_start(
                        ktw[:, :, r_, :],
                        out_bounce[r_, 0, :, 2 * w:2 * w + 2, :])
                    nc.sync.dma_start(
                        vw[:, r_, :, :],
                        vg_view[r_, :, :, 256 * w:256 * w + 256])

                def v_tile(tau):  # [128 keys, 256 dims of this wave]
                    return vw[:, tau // R, tau % R, :]

                pt = {}
                for pair in (2 * w, 2 * w + 1):
                    lp = pair - 2 * w
                    ptA = ptp.tile([P, NKT, SL], BF16, tag="pt")
                    ptB = ptp.tile([P, NKT, SL], BF16, tag="pt")
                    for tg in range(NKT // 2):
                        sA = psB.tile([P, 2, 512], F32, tag="sc")
                        sB = psB.tile([P, 2, 512], F32, tag="sc")
                        for j in (0, 1):
                            tau = 2 * tg + j
                            r_, kl = tau // R, tau % R
                            ks = slice(128 * kl, 128 * kl + 128)
                            nc.tensor.matmul(
                                sA[:, j, :], ktw[0:64, lp, r_, ks],
                                qt_sb[0:64, pair, :],
                                start=True, stop=True, tile_position=(0, 0))
                            nc.tensor.matmul(
                                sB[:, j, :], ktw[64:128, lp, r_, ks],
                                qt_sb[64:128, pair, :],
                                start=True, stop=True, tile_position=(64, 0))
                        nc.scalar.activation(ptA[:, 2 * tg:2 * tg + 2, :],
                                             sA[:, :, :], AF.Exp, scale=SCALE)
                        nc.scalar.activation(ptB[:, 2 * tg:2 * tg + 2, :],
                                             sB[:, :, :], AF.Exp, scale=SCALE)
                    pt[2 * pair] = ptA
                    pt[2 * pair + 1] = ptB

                # P @ V, two heads column-packed per pair
                ctx_ps = {}
                for pair in (2 * w, 2 * w + 1):
                    hA, hB = 2 * pair, 2 * pair + 1
                    gA, gB = hA - 4 * w, hB - 4 * w
                    cp = psA.tile([P, 512], F32, tag="pc")
                    for tau in range(NKT):
                        vt = v_tile(tau)
                        nc.tensor.matmul(
                            cp[0:64, :], vt[:, 64 * gA:64 * gA + 64],
                            pt[hA][:, tau, :],
                            start=(tau == 0), stop=(tau == NKT - 1),
                            tile_position=(0, 0))
                        nc.tensor.matmul(
                            cp[64:128, :], vt[:, 64 * gB:64 * gB + 64],
                            pt[hB][:, tau, :],
                            start=(tau == 0), stop=(tau == NKT - 1),
                            tile_position=(0, 64))
                    ctx_ps[pair] = cp

                # softmax denominators: ones-matmuls, 4 heads column-packed
                sums_ps = psC.tile([P, 512], F32, tag="sums")
                for tau in range(NKT):
                    for g in range(4):
                        h = 4 * w + g
                        nc.tensor.matmul(
                            sums_ps[32 * g:32 * g + 1, :], ones_sb[:, 0:1],
                            pt[h][:, tau, :],
                            start=(tau == 0), stop=(tau == NKT - 1),
                            tile_position=(0, 32 * g))

                # scale = gate / denom, broadcast to the 64 dims of each head
                recip_w = work.tile([P, 512], F32, tag="recip")
                nc.vector.reciprocal(recip_w[:], sums_ps[:])
                gate_al = work.tile([P, 512], F32, tag="gal")
                for g in range(4):
                    h = 4 * w + g
                    nc.sync.dma_start(gate_al[32 * g:32 * g + 1, :],
                                      gate_sb[h:h + 1, :])
                s_w = work.tile([P, 512], F32, tag="sw")
                nc.vector.tensor_mul(s_w[:], gate_al[:], recip_w[:])

                for pair in (2 * w, 2 * w + 1):
                    gA = 2 * (pair - 2 * w)
                    sbcA = work.tile([64, 512], F32, tag="sbc")
                    sbcB = work.tile([64, 512], F32, tag="sbc")
                    sA_t = work.tile([1, 512], F32, tag="srow")
                    sB_t = work.tile([1, 512], F32, tag="srow")
                    nc.sync.dma_start(sA_t[:], s_w[32 * gA:32 * gA + 1, :])
                    nc.sync.dma_start(sB_t[:], s_w[32 * gA + 32:32 * gA + 33, :])
                    nc.gpsimd.partition_broadcast(sbcA[:, :], sA_t[0:1, :])
                    nc.gpsimd.partition_broadcast(sbcB[:, :], sB_t[0:1, :])
                    nc.vector.tensor_mul(ctxT[0:64, pair, :],
                                         ctx_ps[pair][0:64, :], sbcA[:, :])
                    nc.vector.tensor_mul(ctxT[64:128, pair, :],
                                         ctx_ps[pair][64:128, :], sbcB[:, :])

            # ---- output projection --------------------------------------
            wo_sb = load_w(wo)
            bo_sb = cst.tile([P, D], F32, name="bo_sb")
            nc.sync.dma_start(bo_sb[:], bob[:])
            for qi in range(SL // P):
                osb = work.tile([P, D], F32, tag="osb")
                for c2 in range(2):
                    po = psA.tile([P, 512], F32, tag="pc")
                    for pair in range(DT):
                        nc.tensor.matmul(
                            po[:], ctxT[:, pair, 128 * qi:128 * qi + 128],
                            wo_sb[:, pair, 512 * c2:512 * c2 + 512],
                            start=(pair == 0), stop=(pair == DT - 1))
                    nc.vector.tensor_add(osb[:, 512 * c2:512 * c2 + 512],
                                         po[:], bo_sb[:, 512 * c2:512 * c2 + 512])
                nc.sync.dma_start(out[128 * qi:128 * qi + 128, :], osb[:])

    nc.compile()
    return nc


def _prep_inputs(query, key_, value, Wq, bq, Wk, bk, Wv, bv, Wo, bo, Wg, bg):
    """Host-side sharding / layout prep. Returns in_maps for the 8 cores."""
    f32 = np.float32

    def bf(x):
        return np.ascontiguousarray(np.asarray(x, f32)).astype(BF16_NP)

    wq_b, wk_b, wv_b, wo_b, wg_b = bf(Wq), bf(Wk), bf(Wv), bf(Wo), bf(Wg)
    bq_pm = np.ascontiguousarray(np.asarray(bq, f32).reshape(DT, P).T)
    bk_pm = np.ascontiguousarray(np.asarray(bk, f32).reshape(DT, P).T)
    bv_b = np.ascontiguousarray(
        np.broadcast_to(np.asarray(bv, f32), (P, D)))
    bo_b = np.ascontiguousarray(
        np.broadcast_to(np.asarray(bo, f32), (P, D)))
    bg_c = np.ascontiguousarray(np.asarray(bg, f32).reshape(H, 1))

    qT = [np.asarray(query[b], f32).T for b in range(B)]
    kT = [np.asarray(key_[b], f32).T for b in range(B)]
    vT = [np.asarray(value[b], f32).T for b in range(B)]

    in_maps = []
    for c in range(N_CORES):
        b, r = c // R, c % R
        rows = slice(SL * r, SL * (r + 1))
        in_maps.append({
            "xqT": np.ascontiguousarray(qT[b][:, rows]).astype(BF16_NP),
            "xkT": np.ascontiguousarray(kT[b][:, rows]).astype(BF16_NP),
            "xvT": np.ascontiguousarray(vT[b][:, rows]).astype(BF16_NP),
            "wq": wq_b, "wk": wk_b, "wv": wv_b, "wo": wo_b, "wg": wg_b,
            "bq": bq_pm, "bk": bk_pm, "bvb": bv_b, "bob": bo_b, "bg": bg_c,
        })
    return in_maps


def kernel(query, key_, value, Wq, bq, Wk, bk, Wv, bv, Wo, bo, Wg, bg):
    global LAST_EXEC_TIME_NS
    if "nc" not in _CACHE:
        _CACHE["nc"] = _build()
    nc = _CACHE["nc"]

    in_maps = _prep_inputs(query, key_, value, Wq, bq, Wk, bk, Wv, bv,
                           Wo, bo, Wg, bg)
    trace = bool(os.environ.get("BASS_TRACE"))
    res = run_bass_kernel_spmd(nc, in_maps, core_ids=list(range(N_CORES)),
                               trace=trace)
    LAST_EXEC_TIME_NS = res.exec_time_ns

    out = np.empty((B, S, D), np.float32)
    for c in range(N_CORES):
        b, r = c // R, c % R
        out[b, SL * r:SL * (r + 1), :] = res.results[c]["out"]
    return out


# revision 7
# speedup vs baseline: 1.2653x; 1.2653x over previous
"""AdaptiveAttention (B=2, S=2048, D=1024, H=16) on 8 TRN2 NeuronCores.

Sharding: query-parallel. Core c (c = 0..7) owns batch b = c//4 and query rows
[512*(c%4), 512*(c%4+1)). K/V projections are computed for the core's own 512
key rows and AllGathered (bf16) within each batch group of 4 cores. Each core
then computes all 16 heads of attention for its 512 queries against the full
2048 keys, applies the per-(head, query) sigmoid-gate / softmax-denominator
scale to the context, and runs the full output projection for its rows.
The host concatenates the 8 disjoint [512, 1024] output blocks.

On-chip layout is "feature-major" (transposed): projections produce Q^T/K^T
directly so scores come out keys-on-partitions, which feeds exp (ScalarE, with
the 1/sqrt(dk) folded into the activation scale) and the P@V matmul without
any on-device transposes. Matmuls are bf16 with f32 PSUM accumulation; scores
use 2-head row-packing (K=64) and P@V uses 2-head column-packing (M=64);
softmax denominators come from ones-vector matmuls column-packed 4 heads wide.
"""

import contextlib
import ctypes
import os
import sys
import types

import numpy as np
import ml_dtypes


# ---------------------------------------------------------------------------
# NTFF profiling hook shim (antenv.axon_hooks is absent in this image).
# Only used when BASS_TRACE is set; harmless otherwise.
# ---------------------------------------------------------------------------
def _install_ntff_hook_shim():
    if "antenv.axon_hooks" in sys.modules:
        return
    try:
        lib = ctypes.CDLL("/opt/axon/libaxon_pjrt.so")
    except OSError:
        return
    if not hasattr(lib, "axon_start_nrt_profile"):
        return
    lib.axon_start_nrt_profile.argtypes = [
        ctypes.POINTER(ctypes.c_int64),
        ctypes.c_size_t,
    ]
    lib.axon_start_nrt_profile.restype = ctypes.c_int64
    lib.axon_stop_nrt_profile.argtypes = [ctypes.c_char_p]
    lib.axon_stop_nrt_profile.restype = ctypes.c_int64

    @contextlib.contextmanager
    def _hook(output_dir, device_ids):
        import jax

        jax.devices()
        if device_ids:
            ids = (ctypes.c_int64 * len(device_ids))(*device_ids)
            rc = lib.axon_start_nrt_profile(ids, len(device_ids))
        else:
            rc = lib.axon_start_nrt_profile(None, 0)
        if rc != 0:
            raise RuntimeError(f"axon_start_nrt_profile rc={rc}")
        try:
            yield
        finally:
            n = lib.axon_stop_nrt_profile(str(output_dir).encode())
            if n < 0:
                raise RuntimeError(f"axon_stop_nrt_profile rc={n}")

    mod = types.ModuleType("antenv.axon_hooks")
    _state = {"hook": _hook}
    mod.get_axon_ntff_profile_hook = lambda: _state["hook"]
    mod.set_axon_ntff_profile_hook = lambda h: _state.__setitem__("hook", h)
    sys.modules["antenv.axon_hooks"] = mod
    try:
        import antenv

        antenv.axon_hooks = mod
    except ImportError:
        pass


_install_ntff_hook_shim()

import concourse.bass as bass  # noqa: E402
import concourse.mybir as mybir  # noqa: E402
import concourse.tile as tile  # noqa: E402
from concourse import bacc  # noqa: E402
from concourse.bass_utils import run_bass_kernel_spmd  # noqa: E402

# ---------------------------------------------------------------------------
# Problem constants (hardcoded; kernel.py must be self-contained)
# ---------------------------------------------------------------------------
B, S, D, H = 2, 2048, 1024, 16
DK = D // H                  # 64
N_CORES = 8
R = 4                        # ranks per batch group
SL = S // R                  # 512 local rows per core
P = 128
DT = D // P                  # 8 feature tiles
NKT = S // P                 # 16 key tiles
SCALE = DK ** -0.5

F32 = mybir.dt.float32
BF16 = mybir.dt.bfloat16
AF = mybir.ActivationFunctionType
BF16_NP = ml_dtypes.bfloat16

_CACHE = {}
LAST_EXEC_TIME_NS = None


def _build():
    nc = bacc.Bacc("TRN2", target_bir_lowering=False, debug=False,
                   num_devices=N_CORES)

    # ---- I/O --------------------------------------------------------------
    xqT = nc.dram_tensor("xqT", [D, SL], BF16, kind="ExternalInput")
    xkT = nc.dram_tensor("xkT", [D, SL], BF16, kind="ExternalInput")
    xvT = nc.dram_tensor("xvT", [D, SL], BF16, kind="ExternalInput")
    wq = nc.dram_tensor("wq", [D, D], BF16, kind="ExternalInput")
    wk = nc.dram_tensor("wk", [D, D], BF16, kind="ExternalInput")
    wv = nc.dram_tensor("wv", [D, D], BF16, kind="ExternalInput")
    wo = nc.dram_tensor("wo", [D, D], BF16, kind="ExternalInput")
    wg = nc.dram_tensor("wg", [D, H], BF16, kind="ExternalInput")
    bq = nc.dram_tensor("bq", [P, DT], F32, kind="ExternalInput")
    bk = nc.dram_tensor("bk", [P, DT], F32, kind="ExternalInput")
    bvb = nc.dram_tensor("bvb", [P, D], F32, kind="ExternalInput")
    bob = nc.dram_tensor("bob", [P, D], F32, kind="ExternalInput")
    bg = nc.dram_tensor("bg", [H, 1], F32, kind="ExternalInput")
    out = nc.dram_tensor("out", [SL, D], F32, kind="ExternalOutput")

    with tile.TileContext(nc) as tc:
        with (
            tc.tile_pool(name="cst", bufs=1) as cst,
            tc.tile_pool(name="wpool", bufs=2) as wpool,
            tc.tile_pool(name="xpool", bufs=2) as xpool,
            tc.tile_pool(name="kvpool", bufs=4) as kvpool,
            tc.tile_pool(name="work", bufs=2) as work,
            tc.tile_pool(name="pt_pool", bufs=4) as ptp,
            tc.tile_pool(name="psA", bufs=2, space="PSUM") as psA,
            tc.tile_pool(name="psB", bufs=2, space="PSUM") as psB,
            tc.tile_pool(name="psC", bufs=2, space="PSUM") as psC,
            tc.tile_pool(name="dram", bufs=1, space="DRAM") as dram,
        ):
            # ---- load weights / inputs -----------------------------------
            def load_w(dram_t):  # rotating weight slot [128, 8, 1024]
                t = wpool.tile([P, DT, D], BF16, tag="wmat")
                nc.sync.dma_start(
                    t[:], dram_t.ap().rearrange("(t p) f -> p t f", p=P))
                return t

            def load_x(dram_t):  # rotating activation slot [128, 8, 512]
                t = xpool.tile([P, DT, SL], BF16, tag="xmat")
                nc.sync.dma_start(
                    t[:], dram_t.ap().rearrange("(t p) f -> p t f", p=P))
                return t

            wk_sb = load_w(wk)
            wv_sb = load_w(wv)
            xk_sb = load_x(xkT)
            xv_sb = load_x(xvT)
            bk_sb = cst.tile([P, DT], F32, name="bk_sb")
            nc.sync.dma_start(bk_sb[:], bk[:])
            bv_sb = cst.tile([P, D], F32, name="bv_sb")
            nc.sync.dma_start(bv_sb[:], bvb[:])

            # ---- K^T / V projections + per-wave pipelined AllGather ------
            # piece w carries K^T dims-tiles {2w, 2w+1} and V dim-window
            # [256w, 256w+256) for the core's 512 local keys (512KB/rank).
            ktloc = kvpool.tile([P, DT, SL], BF16, tag="kv")
            vloc = kvpool.tile([P, DT, SL], BF16, tag="kv")
            vloc_v = vloc[:].rearrange("p t k -> p (t k)").rearrange(
                "p (a d) -> p a d", a=R)
            in_b = [dram.tile([2, P, 2 * SL], BF16, name=f"in_b{i}")
                    for i in range(4)]
            out_b = [dram.tile([R, 2, P, 2 * SL], BF16, name=f"out_b{i}")
                     for i in range(4)]

            def k_proj(mt):
                pp = psA.tile([P, 512], F32, tag="pc")
                for kt in range(DT):
                    nc.tensor.matmul(pp[:], wk_sb[:, kt, 128 * mt:128 * mt + 128],
                                     xk_sb[:, kt, :],
                                     start=(kt == 0), stop=(kt == DT - 1))
                nc.vector.tensor_scalar_add(ktloc[:, mt, :], pp[:],
                                            bk_sb[:, mt:mt + 1])

            def v_proj(kb, c2):
                pp = psA.tile([P, 512], F32, tag="pc")
                for kt in range(DT):
                    nc.tensor.matmul(
                        pp[:], xv_sb[:, kt, 128 * kb:128 * kb + 128],
                        wv_sb[:, kt, 512 * c2:512 * c2 + 512],
                        start=(kt == 0), stop=(kt == DT - 1))
                nc.vector.tensor_add(vloc_v[:, kb, 512 * c2:512 * c2 + 512],
                                     pp[:], bv_sb[:, 512 * c2:512 * c2 + 512])

            def issue_piece(w):
                # bounce + collective for wave w's K/V piece
                nc.sync.dma_start(
                    in_b[w][0].rearrange("p (t k) -> p t k", t=2),
                    ktloc[:, 2 * w:2 * w + 2, :])
                nc.sync.dma_start(
                    in_b[w][1].rearrange("p (a d) -> p a d", a=R),
                    vloc_v[:, :, 256 * w:256 * w + 256])
                nc.gpsimd.collective_compute(
                    "AllGather",
                    mybir.AluOpType.bypass,
                    replica_groups=[[0, 1, 2, 3], [4, 5, 6, 7]],
                    ins=[in_b[w].opt()],
                    outs=[out_b[w].opt()],
                )

            # emission order: finish the data for piece w, then trigger its
            # AllGather, so collectives pipeline behind the projections.
            k_proj(0)
            k_proj(1)
            for kb in range(R):
                v_proj(kb, 0)
            issue_piece(0)
            k_proj(2)
            k_proj(3)
            issue_piece(1)
            k_proj(4)
            k_proj(5)
            for kb in range(R):
                v_proj(kb, 1)
            issue_piece(2)
            k_proj(6)
            k_proj(7)
            issue_piece(3)

            # ---- Q^T projection + gate (overlaps the AllGather) ----------
            wq_sb = load_w(wq)
            xq_sb = load_x(xqT)
            wg_sb = cst.tile([P, DT, H], BF16, name="wg_sb")
            nc.sync.dma_start(
                wg_sb[:], wg.ap().rearrange("(t p) h -> p t h", p=P))
            bq_sb = cst.tile([P, DT], F32, name="bq_sb")
            nc.sync.dma_start(bq_sb[:], bq[:])
            bg_sb = cst.tile([H, 1], F32, name="bg_sb")
            nc.sync.dma_start(bg_sb[:], bg[:])

            qt_sb = cst.tile([P, DT, SL], BF16, name="qt_sb")
            for mt in range(DT):
                pp = psA.tile([P, 512], F32, tag="pc")
                for kt in range(DT):
                    nc.tensor.matmul(pp[:], wq_sb[:, kt, 128 * mt:128 * mt + 128],
                                     xq_sb[:, kt, :],
                                     start=(kt == 0), stop=(kt == DT - 1))
                nc.vector.tensor_scalar_add(qt_sb[:, mt, :], pp[:],
                                            bq_sb[:, mt:mt + 1])

            gate_sb = cst.tile([H, SL], F32, name="gate_sb")
            gp = psC.tile([H, 512], F32, tag="sums")
            for kt in range(DT):
                nc.tensor.matmul(gp[:], wg_sb[:, kt, :], xq_sb[:, kt, :],
                                 start=(kt == 0), stop=(kt == DT - 1))
            nc.scalar.activation(gate_sb[:], gp[:], AF.Sigmoid,
                                 bias=bg_sb[:, 0:1])

            ones_sb = cst.tile([P, 1], BF16, name="ones_sb")
            nc.vector.memset(ones_sb[:], 1.0)

            # ---- attention waves (4 heads per wave) ----------------------
            ctxT = cst.tile([P, DT, SL], BF16, name="ctxT")
            for w in range(4):
                # stream this wave's K^T dims-tiles and V dim-window
                ktw = kvpool.tile([P, 2, R, SL], BF16, tag="kv")
                vw = kvpool.tile([P, R, R, 256], BF16, tag="kv")
                for r_ in range(R):
                    nc.sync.dma_start(
                        ktw[:, :, r_, :],
                        out_b[w][r_, 0].rearrange("p (t k) -> p t k", t=2))
                    nc.sync.dma_start(
                        vw[:, r_, :, :],
                        out_b[w][r_, 1].rearrange("p (a d) -> p a d", a=R))

                def v_tile(tau):  # [128 keys, 256 dims of this wave]
                    return vw[:, tau // R, tau % R, :]

                pt = {}
                for pair in (2 * w, 2 * w + 1):
                    lp = pair - 2 * w
                    ptA = ptp.tile([P, NKT, SL], BF16, tag="pt")
                    ptB = ptp.tile([P, NKT, SL], BF16, tag="pt")
                    for tg in range(NKT // 2):
                        sA = psB.tile([P, 2, 512], F32, tag="sc")
                        sB = psB.tile([P, 2, 512], F32, tag="sc")
                        for j in (0, 1):
                            tau = 2 * tg + j
                            r_, kl = tau // R, tau % R
                            klo = slice(128 * kl, 128 * kl + 64)
                            khi = slice(128 * kl + 64, 128 * kl + 128)
                            nc.tensor.matmul(
                                sA[0:64, j, :], ktw[0:64, lp, r_, klo],
                                qt_sb[0:64, pair, :],
                                start=True, stop=True, tile_position=(0, 0))
                            nc.tensor.matmul(
                                sA[64:128, j, :], ktw[0:64, lp, r_, khi],
                                qt_sb[0:64, pair, :],
                                start=True, stop=True, tile_position=(0, 64))
                            nc.tensor.matmul(
                                sB[0:64, j, :], ktw[64:128, lp, r_, klo],
                                qt_sb[64:128, pair, :],
                                start=True, stop=True, tile_position=(64, 0))
                            nc.tensor.matmul(
                                sB[64:128, j, :], ktw[64:128, lp, r_, khi],
                                qt_sb[64:128, pair, :],
                                start=True, stop=True, tile_position=(64, 64))
                        nc.scalar.activation(ptA[:, 2 * tg:2 * tg + 2, :],
                                             sA[:, :, :], AF.Exp, scale=SCALE)
                        nc.scalar.activation(ptB[:, 2 * tg:2 * tg + 2, :],
                                             sB[:, :, :], AF.Exp, scale=SCALE)
                    pt[2 * pair] = ptA
                    pt[2 * pair + 1] = ptB

                # P @ V, two heads column-packed per pair
                ctx_ps = {}
                for pair in (2 * w, 2 * w + 1):
                    hA, hB = 2 * pair, 2 * pair + 1
                    gA, gB = hA - 4 * w, hB - 4 * w
                    cp = psA.tile([P, 512], F32, tag="pc")
                    for tau in range(NKT):
                        vt = v_tile(tau)
                        nc.tensor.matmul(
                            cp[0:64, :], vt[:, 64 * gA:64 * gA + 64],
                            pt[hA][:, tau, :],
                            start=(tau == 0), stop=(tau == NKT - 1),
                            tile_position=(0, 0))
                        nc.tensor.matmul(
                            cp[64:128, :], vt[:, 64 * gB:64 * gB + 64],
                            pt[hB][:, tau, :],
                            start=(tau == 0), stop=(tau == NKT - 1),
                            tile_position=(0, 64))
                    ctx_ps[pair] = cp

                # softmax denominators: ones-matmuls, 4 heads column-packed
                sums_ps = psC.tile([P, 512], F32, tag="sums")
                for tau in range(NKT):
                    for g in range(4):
                        h = 4 * w + g
                        nc.tensor.matmul(
                            sums_ps[32 * g:32 * g + 1, :], ones_sb[:, 0:1],
                            pt[h][:, tau, :],
                            start=(tau == 0), stop=(tau == NKT - 1),
                            tile_position=(0, 32 * g))

                # scale = gate / denom, broadcast to the 64 dims of each head
                recip_w = work.tile([P, 512], F32, tag="recip")
                nc.vector.reciprocal(recip_w[:], sums_ps[:])
                gate_al = work.tile([P, 512], F32, tag="gal")
                for g in range(4):
                    h = 4 * w + g
                    nc.sync.dma_start(gate_al[32 * g:32 * g + 1, :],
                                      gate_sb[h:h + 1, :])
                s_w = work.tile([P, 512], F32, tag="sw")
                nc.vector.tensor_mul(s_w[:], gate_al[:], recip_w[:])

                for pair in (2 * w, 2 * w + 1):
                    gA = 2 * (pair - 2 * w)
                    sbcA = work.tile([64, 512], F32, tag="sbc")
                    sbcB = work.tile([64, 512], F32, tag="sbc")
                    sA_t = work.tile([1, 512], F32, tag="srow")
                    sB_t = work.tile([1, 512], F32, tag="srow")
                    nc.sync.dma_start(sA_t[:], s_w[32 * gA:32 * gA + 1, :])
                    nc.sync.dma_start(sB_t[:], s_w[32 * gA + 32:32 * gA + 33, :])
                    nc.gpsimd.partition_broadcast(sbcA[:, :], sA_t[0:1, :])
                    nc.gpsimd.partition_broadcast(sbcB[:, :], sB_t[0:1, :])
                    nc.vector.tensor_mul(ctxT[0:64, pair, :],
                                         ctx_ps[pair][0:64, :], sbcA[:, :])
                    nc.vector.tensor_mul(ctxT[64:128, pair, :],
                                         ctx_ps[pair][64:128, :], sbcB[:, :])

            # ---- output projection --------------------------------------
            wo_sb = load_w(wo)
            bo_sb = cst.tile([P, D], F32, name="bo_sb")
            nc.sync.dma_start(bo_sb[:], bob[:])
            for qi in range(SL // P):
                osb = work.tile([P, D], F32, tag="osb")
                for c2 in range(2):
                    po = psA.tile([P, 512], F32, tag="pc")
                    for pair in range(DT):
                        nc.tensor.matmul(
                            po[:], ctxT[:, pair, 128 * qi:128 * qi + 128],
                            wo_sb[:, pair, 512 * c2:512 * c2 + 512],
                            start=(pair == 0), stop=(pair == DT - 1))
                    nc.vector.tensor_add(osb[:, 512 * c2:512 * c2 + 512],
                                         po[:], bo_sb[:, 512 * c2:512 * c2 + 512])
                nc.sync.dma_start(out[128 * qi:128 * qi + 128, :], osb[:])

    nc.compile()
    return nc


def _prep_inputs(query, key_, value, Wq, bq, Wk, bk, Wv, bv, Wo, bo, Wg, bg):
    """Host-side sharding / layout prep. Returns in_maps for the 8 cores."""
    f32 = np.float32

    def bf(x):
        return np.ascontiguousarray(np.asarray(x, f32)).astype(BF16_NP)

    wq_b, wk_b, wv_b, wo_b, wg_b = bf(Wq), bf(Wk), bf(Wv), bf(Wo), bf(Wg)
    bq_pm = np.ascontiguousarray(np.asarray(bq, f32).reshape(DT, P).T)
    bk_pm = np.ascontiguousarray(np.asarray(bk, f32).reshape(DT, P).T)
    bv_b = np.ascontiguousarray(
        np.broadcast_to(np.asarray(bv, f32), (P, D)))
    bo_b = np.ascontiguousarray(
        np.broadcast_to(np.asarray(bo, f32), (P, D)))
    bg_c = np.ascontiguousarray(np.asarray(bg, f32).reshape(H, 1))

    qT = [np.asarray(query[b], f32).T for b in range(B)]
    kT = [np.asarray(key_[b], f32).T for b in range(B)]
    vT = [np.asarray(value[b], f32).T for b in range(B)]

    in_maps = []
    for c in range(N_CORES):
        b, r = c // R, c % R
        rows = slice(SL * r, SL * (r + 1))
        in_maps.append({
            "xqT": np.ascontiguousarray(qT[b][:, rows]).astype(BF16_NP),
            "xkT": np.ascontiguousarray(kT[b][:, rows]).astype(BF16_NP),
            "xvT": np.ascontiguousarray(vT[b][:, rows]).astype(BF16_NP),
            "wq": wq_b, "wk": wk_b, "wv": wv_b, "wo": wo_b, "wg": wg_b,
            "bq": bq_pm, "bk": bk_pm, "bvb": bv_b, "bob": bo_b, "bg": bg_c,
        })
    return in_maps


def kernel(query, key_, value, Wq, bq, Wk, bk, Wv, bv, Wo, bo, Wg, bg):
    global LAST_EXEC_TIME_NS
    if "nc" not in _CACHE:
        _CACHE["nc"] = _build()
    nc = _CACHE["nc"]

    in_maps = _prep_inputs(query, key_, value, Wq, bq, Wk, bk, Wv, bv,
                           Wo, bo, Wg, bg)
    trace = bool(os.environ.get("BASS_TRACE"))
    res = run_bass_kernel_spmd(nc, in_maps, core_ids=list(range(N_CORES)),
                               trace=trace)
    LAST_EXEC_TIME_NS = res.exec_time_ns

    out = np.empty((B, S, D), np.float32)
    for c in range(N_CORES):
        b, r = c // R, c % R
        out[b, SL * r:SL * (r + 1), :] = res.results[c]["out"]
    return out


# revision 10
# speedup vs baseline: 1.2716x; 1.0049x over previous
"""AdaptiveAttention (B=2, S=2048, D=1024, H=16) on 8 TRN2 NeuronCores.

Sharding: query-parallel. Core c (c = 0..7) owns batch b = c//4 and query rows
[512*(c%4), 512*(c%4+1)). K/V projections are computed for the core's own 512
key rows and AllGathered (bf16) within each batch group of 4 cores. Each core
then computes all 16 heads of attention for its 512 queries against the full
2048 keys, applies the per-(head, query) sigmoid-gate / softmax-denominator
scale to the context, and runs the full output projection for its rows.
The host concatenates the 8 disjoint [512, 1024] output blocks.

On-chip layout is "feature-major" (transposed): projections produce Q^T/K^T
directly so scores come out keys-on-partitions, which feeds exp (ScalarE, with
the 1/sqrt(dk) folded into the activation scale) and the P@V matmul without
any on-device transposes. Matmuls are bf16 with f32 PSUM accumulation; scores
use 2-head row-packing (K=64) and P@V uses 2-head column-packing (M=64);
softmax denominators come from ones-vector matmuls column-packed 4 heads wide.
"""

import contextlib
import ctypes
import os
import sys
import types

import numpy as np
import ml_dtypes


# ---------------------------------------------------------------------------
# NTFF profiling hook shim (antenv.axon_hooks is absent in this image).
# Only used when BASS_TRACE is set; harmless otherwise.
# ---------------------------------------------------------------------------
def _install_ntff_hook_shim():
    if "antenv.axon_hooks" in sys.modules:
        return
    try:
        lib = ctypes.CDLL("/opt/axon/libaxon_pjrt.so")
    except OSError:
        return
    if not hasattr(lib, "axon_start_nrt_profile"):
        return
    lib.axon_start_nrt_profile.argtypes = [
        ctypes.POINTER(ctypes.c_int64),
        ctypes.c_size_t,
    ]
    lib.axon_start_nrt_profile.restype = ctypes.c_int64
    lib.axon_stop_nrt_profile.argtypes = [ctypes.c_char_p]
    lib.axon_stop_nrt_profile.restype = ctypes.c_int64

    @contextlib.contextmanager
    def _hook(output_dir, device_ids):
        import jax

        jax.devices()
        if device_ids:
            ids = (ctypes.c_int64 * len(device_ids))(*device_ids)
            rc = lib.axon_start_nrt_profile(ids, len(device_ids))
        else:
            rc = lib.axon_start_nrt_profile(None, 0)
        if rc != 0:
            raise RuntimeError(f"axon_start_nrt_profile rc={rc}")
        try:
            yield
        finally:
            n = lib.axon_stop_nrt_profile(str(output_dir).encode())
            if n < 0:
                raise RuntimeError(f"axon_stop_nrt_profile rc={n}")

    mod = types.ModuleType("antenv.axon_hooks")
    _state = {"hook": _hook}
    mod.get_axon_ntff_profile_hook = lambda: _state["hook"]
    mod.set_axon_ntff_profile_hook = lambda h: _state.__setitem__("hook", h)
    sys.modules["antenv.axon_hooks"] = mod
    try:
        import antenv

        antenv.axon_hooks = mod
    except ImportError:
        pass


_install_ntff_hook_shim()

import concourse.bass as bass  # noqa: E402
import concourse.mybir as mybir  # noqa: E402
import concourse.tile as tile  # noqa: E402
from concourse import bacc  # noqa: E402
from concourse.bass_utils import run_bass_kernel_spmd  # noqa: E402

# ---------------------------------------------------------------------------
# Problem constants (hardcoded; kernel.py must be self-contained)
# ---------------------------------------------------------------------------
B, S, D, H = 2, 2048, 1024, 16
DK = D // H                  # 64
N_CORES = 8
R = 4                        # ranks per batch group
SL = S // R                  # 512 local rows per core
P = 128
DT = D // P                  # 8 feature tiles
NKT = S // P                 # 16 key tiles
SCALE = DK ** -0.5

F32 = mybir.dt.float32
BF16 = mybir.dt.bfloat16
AF = mybir.ActivationFunctionType
BF16_NP = ml_dtypes.bfloat16

_CACHE = {}
LAST_EXEC_TIME_NS = None


def _build():
    nc = bacc.Bacc("TRN2", target_bir_lowering=False, debug=False,
                   num_devices=N_CORES)

    # ---- I/O --------------------------------------------------------------
    xqT = nc.dram_tensor("xqT", [D, SL], BF16, kind="ExternalInput")
    xkT = nc.dram_tensor("xkT", [D, SL], BF16, kind="ExternalInput")
    xvT = nc.dram_tensor("xvT", [D, SL], BF16, kind="ExternalInput")
    wq = nc.dram_tensor("wq", [D, D], BF16, kind="ExternalInput")
    wk = nc.dram_tensor("wk", [D, D], BF16, kind="ExternalInput")
    wv = nc.dram_tensor("wv", [D, D], BF16, kind="ExternalInput")
    wo = nc.dram_tensor("wo", [D, D], BF16, kind="ExternalInput")
    wg = nc.dram_tensor("wg", [P, DT, H], BF16, kind="ExternalInput")
    bq = nc.dram_tensor("bq", [P, DT], F32, kind="ExternalInput")
    bk = nc.dram_tensor("bk", [P, DT], F32, kind="ExternalInput")
    bvb = nc.dram_tensor("bvb", [P, D], BF16, kind="ExternalInput")
    bob = nc.dram_tensor("bob", [P, D], BF16, kind="ExternalInput")
    bg = nc.dram_tensor("bg", [H, 1], F32, kind="ExternalInput")
    out = nc.dram_tensor("out", [SL, D], F32, kind="ExternalOutput")

    with tile.TileContext(nc) as tc:
        with (
            tc.tile_pool(name="cst", bufs=1) as cst,
            tc.tile_pool(name="wpool", bufs=2) as wpool,
            tc.tile_pool(name="xpool", bufs=2) as xpool,
            tc.tile_pool(name="kvpool", bufs=5) as kvpool,
            tc.tile_pool(name="work", bufs=2) as work,
            tc.tile_pool(name="pt_pool", bufs=4) as ptp,
            tc.tile_pool(name="psA", bufs=2, space="PSUM") as psA,
            tc.tile_pool(name="psB", bufs=2, space="PSUM") as psB,
            tc.tile_pool(name="psC", bufs=2, space="PSUM") as psC,
            tc.tile_pool(name="dram", bufs=1, space="DRAM") as dram,
        ):
            # ---- load weights / inputs -----------------------------------
            # chunked DMAs (2 dims-tiles per chunk) so the first projection
            # matmuls start as soon as their slices land.
            def load_w(dram_t):  # rotating weight slot [128, 8, 1024]
                t = wpool.tile([P, DT, D], BF16, tag="wmat")
                src3 = dram_t.ap().rearrange("(t p) f -> p t f", p=P)
                for c in range(4):
                    nc.sync.dma_start(t[:, 2 * c:2 * c + 2, :],
                                      src3[:, 2 * c:2 * c + 2, :])
                return t

            def load_x(dram_t):  # rotating activation slot [128, 8, 512]
                t = xpool.tile([P, DT, SL], BF16, tag="xmat")
                src3 = dram_t.ap().rearrange("(t p) f -> p t f", p=P)
                for c in range(4):
                    nc.sync.dma_start(t[:, 2 * c:2 * c + 2, :],
                                      src3[:, 2 * c:2 * c + 2, :])
                return t

            wk_sb = load_w(wk)
            xk_sb = load_x(xkT)
            wv_sb = load_w(wv)
            xv_sb = load_x(xvT)
            bk_sb = cst.tile([P, DT], F32, name="bk_sb")
            nc.sync.dma_start(bk_sb[:], bk[:])
            bv_sb = cst.tile([P, D], BF16, name="bv_sb")
            nc.sync.dma_start(bv_sb[:], bvb[:])

            # ---- K^T / V projections + per-wave pipelined AllGather ------
            # piece w carries K^T dims-tiles {2w, 2w+1} and V dim-window
            # [256w, 256w+256) for the core's 512 local keys (512KB/rank).
            ktloc = kvpool.tile([P, DT, SL], BF16, tag="kv")
            vloc = kvpool.tile([P, DT, SL], BF16, tag="kv")
            vloc_v = vloc[:].rearrange("p t k -> p (t k)").rearrange(
                "p (a d) -> p a d", a=R)
            in_b = [dram.tile([2, P, 2 * SL], BF16, name=f"in_b{i}")
                    for i in range(4)]
            out_b = [dram.tile([R, 2, P, 2 * SL], BF16, name=f"out_b{i}")
                     for i in range(4)]

            def k_proj(mt):
                pp = psA.tile([P, 512], F32, tag="pc")
                for kt in range(DT):
                    nc.tensor.matmul(pp[:], wk_sb[:, kt, 128 * mt:128 * mt + 128],
                                     xk_sb[:, kt, :],
                                     start=(kt == 0), stop=(kt == DT - 1))
                nc.vector.tensor_scalar_add(ktloc[:, mt, :], pp[:],
                                            bk_sb[:, mt:mt + 1])

            def v_proj(kb, c2):
                pp = psA.tile([P, 512], F32, tag="pc")
                for kt in range(DT):
                    nc.tensor.matmul(
                        pp[:], xv_sb[:, kt, 128 * kb:128 * kb + 128],
                        wv_sb[:, kt, 512 * c2:512 * c2 + 512],
                        start=(kt == 0), stop=(kt == DT - 1))
                nc.vector.tensor_add(vloc_v[:, kb, 512 * c2:512 * c2 + 512],
                                     pp[:], bv_sb[:, 512 * c2:512 * c2 + 512])

            def issue_piece(w):
                # bounce + collective for wave w's K/V piece
                nc.sync.dma_start(
                    in_b[w][0].rearrange("p (t k) -> p t k", t=2),
                    ktloc[:, 2 * w:2 * w + 2, :])
                nc.sync.dma_start(
                    in_b[w][1].rearrange("p (a d) -> p a d", a=R),
                    vloc_v[:, :, 256 * w:256 * w + 256])
                nc.gpsimd.collective_compute(
                    "AllGather",
                    mybir.AluOpType.bypass,
                    replica_groups=[[0, 1, 2, 3], [4, 5, 6, 7]],
                    ins=[in_b[w].opt()],
                    outs=[out_b[w].opt()],
                )

            # emission order: finish the data for piece w, then trigger its
            # AllGather, so collectives pipeline behind the projections.
            k_proj(0)
            k_proj(1)
            for kb in range(R):
                v_proj(kb, 0)
            issue_piece(0)
            k_proj(2)
            k_proj(3)
            issue_piece(1)
            k_proj(4)
            k_proj(5)
            for kb in range(R):
                v_proj(kb, 1)
            issue_piece(2)
            k_proj(6)
            k_proj(7)
            issue_piece(3)

            # Q^T projection + gate, overlapping the in-flight AllGathers
            wq_sb = load_w(wq)
            xq_sb = load_x(xqT)
            wg_sb = cst.tile([P, DT, H], BF16, name="wg_sb")
            nc.sync.dma_start(wg_sb[:], wg[:])
            bq_sb = cst.tile([P, DT], F32, name="bq_sb")
            nc.sync.dma_start(bq_sb[:], bq[:])
            bg_sb = cst.tile([H, 1], F32, name="bg_sb")
            nc.sync.dma_start(bg_sb[:], bg[:])

            qt_sb = cst.tile([P, DT, SL], BF16, name="qt_sb")
            for mt in range(DT):
                pp = psA.tile([P, 512], F32, tag="pc")
                for kt in range(DT):
                    nc.tensor.matmul(pp[:], wq_sb[:, kt, 128 * mt:128 * mt + 128],
                                     xq_sb[:, kt, :],
                                     start=(kt == 0), stop=(kt == DT - 1))
                nc.vector.tensor_scalar_add(qt_sb[:, mt, :], pp[:],
                                            bq_sb[:, mt:mt + 1])

            gate_sb = cst.tile([H, SL], F32, name="gate_sb")
            gp = psC.tile([H, 512], F32, tag="sums")
            for kt in range(DT):
                nc.tensor.matmul(gp[:], wg_sb[:, kt, :], xq_sb[:, kt, :],
                                 start=(kt == 0), stop=(kt == DT - 1))
            nc.scalar.activation(gate_sb[:], gp[:], AF.Sigmoid,
                                 bias=bg_sb[:, 0:1])

            ones_sb = cst.tile([P, 1], BF16, name="ones_sb")
            nc.vector.memset(ones_sb[:], 1.0)

            # ---- attention waves (4 heads per wave) ----------------------
            ctxT = cst.tile([P, DT, SL], BF16, name="ctxT")
            for w in range(4):
                # stream this wave's K^T dims-tiles and V dim-window
                ktw = kvpool.tile([P, 2, R, SL], BF16, tag="kv")
                vw = kvpool.tile([P, R, R, 256], BF16, tag="kv")
                for r_ in range(R):
                    nc.sync.dma_start(
                        ktw[:, :, r_, :],
                        out_b[w][r_, 0].rearrange("p (t k) -> p t k", t=2))
                    nc.sync.dma_start(
                        vw[:, r_, :, :],
                        out_b[w][r_, 1].rearrange("p (a d) -> p a d", a=R))

                def v_tile(tau):  # [128 keys, 256 dims of this wave]
                    return vw[:, tau // R, tau % R, :]

                pt = {}
                for pair in (2 * w, 2 * w + 1):
                    lp = pair - 2 * w
                    ptA = ptp.tile([P, NKT, SL], BF16, tag="pt")
                    ptB = ptp.tile([P, NKT, SL], BF16, tag="pt")
                    for tg in range(NKT // 2):
                        sA = psB.tile([P, 2, 512], F32, tag="sc")
                        sB = psB.tile([P, 2, 512], F32, tag="sc")
                        for j in (0, 1):
                            tau = 2 * tg + j
                            r_, kl = tau // R, tau % R
                            klo = slice(128 * kl, 128 * kl + 64)
                            khi = slice(128 * kl + 64, 128 * kl + 128)
                            nc.tensor.matmul(
                                sA[0:64, j, :], ktw[0:64, lp, r_, klo],
                                qt_sb[0:64, pair, :],
                                start=True, stop=True, tile_position=(0, 0))
                            nc.tensor.matmul(
                                sA[64:128, j, :], ktw[0:64, lp, r_, khi],
                                qt_sb[0:64, pair, :],
                                start=True, stop=True, tile_position=(0, 64))
                            nc.tensor.matmul(
                                sB[0:64, j, :], ktw[64:128, lp, r_, klo],
                                qt_sb[64:128, pair, :],
                                start=True, stop=True, tile_position=(64, 0))
                            nc.tensor.matmul(
                                sB[64:128, j, :], ktw[64:128, lp, r_, khi],
                                qt_sb[64:128, pair, :],
                                start=True, stop=True, tile_position=(64, 64))
                        nc.scalar.activation(ptA[:, 2 * tg:2 * tg + 2, :],
                                             sA[:, :, :], AF.Exp, scale=SCALE)
                        nc.scalar.activation(ptB[:, 2 * tg:2 * tg + 2, :],
                                             sB[:, :, :], AF.Exp, scale=SCALE)
                    pt[2 * pair] = ptA
                    pt[2 * pair + 1] = ptB

                # P @ V, two heads column-packed per pair
                ctx_ps = {}
                for pair in (2 * w, 2 * w + 1):
                    hA, hB = 2 * pair, 2 * pair + 1
                    gA, gB = hA - 4 * w, hB - 4 * w
                    cp = psA.tile([P, 512], F32, tag="pc")
                    for tau in range(NKT):
                        vt = v_tile(tau)
                        nc.tensor.matmul(
                            cp[0:64, :], vt[:, 64 * gA:64 * gA + 64],
                            pt[hA][:, tau, :],
                            start=(tau == 0), stop=(tau == NKT - 1),
                            tile_position=(0, 0))
                        nc.tensor.matmul(
                            cp[64:128, :], vt[:, 64 * gB:64 * gB + 64],
                            pt[hB][:, tau, :],
                            start=(tau == 0), stop=(tau == NKT - 1),
                            tile_position=(0, 64))
                    ctx_ps[pair] = cp

                # softmax denominators: ones-matmuls, 4 heads column-packed
                sums_ps = psC.tile([P, 512], F32, tag="sums")
                for tau in range(NKT):
                    for g in range(4):
                        h = 4 * w + g
                        nc.tensor.matmul(
                            sums_ps[32 * g:32 * g + 1, :], ones_sb[:, 0:1],
                            pt[h][:, tau, :],
                            start=(tau == 0), stop=(tau == NKT - 1),
                            tile_position=(0, 32 * g))

                # scale = gate / denom, broadcast to the 64 dims of each head
                recip_w = work.tile([P, 512], F32, tag="recip")
                nc.vector.reciprocal(recip_w[:], sums_ps[:])
                gate_al = work.tile([P, 512], F32, tag="gal")
                for g in range(4):
                    h = 4 * w + g
                    nc.sync.dma_start(gate_al[32 * g:32 * g + 1, :],
                                      gate_sb[h:h + 1, :])
                s_w = work.tile([P, 512], F32, tag="sw")
                nc.vector.tensor_mul(s_w[:], gate_al[:], recip_w[:])

                for pair in (2 * w, 2 * w + 1):
                    gA = 2 * (pair - 2 * w)
                    sbcA = work.tile([64, 512], F32, tag="sbc")
                    sbcB = work.tile([64, 512], F32, tag="sbc")
                    sA_t = work.tile([1, 512], F32, tag="srow")
                    sB_t = work.tile([1, 512], F32, tag="srow")
                    nc.sync.dma_start(sA_t[:], s_w[32 * gA:32 * gA + 1, :])
                    nc.sync.dma_start(sB_t[:], s_w[32 * gA + 32:32 * gA + 33, :])
                    nc.gpsimd.partition_broadcast(sbcA[:, :], sA_t[0:1, :])
                    nc.gpsimd.partition_broadcast(sbcB[:, :], sB_t[0:1, :])
                    nc.vector.tensor_mul(ctxT[0:64, pair, :],
                                         ctx_ps[pair][0:64, :], sbcA[:, :])
                    nc.vector.tensor_mul(ctxT[64:128, pair, :],
                                         ctx_ps[pair][64:128, :], sbcB[:, :])

            # ---- output projection --------------------------------------
            wo_sb = load_w(wo)
            bo_sb = cst.tile([P, D], BF16, name="bo_sb")
            nc.sync.dma_start(bo_sb[:], bob[:])
            for qi in range(SL // P):
                osb = work.tile([P, D], F32, tag="osb")
                for c2 in range(2):
                    po = psA.tile([P, 512], F32, tag="pc")
                    for pair in range(DT):
                        nc.tensor.matmul(
                            po[:], ctxT[:, pair, 128 * qi:128 * qi + 128],
                            wo_sb[:, pair, 512 * c2:512 * c2 + 512],
                            start=(pair == 0), stop=(pair == DT - 1))
                    nc.vector.tensor_add(osb[:, 512 * c2:512 * c2 + 512],
                                         po[:], bo_sb[:, 512 * c2:512 * c2 + 512])
                nc.sync.dma_start(out[128 * qi:128 * qi + 128, :], osb[:])

    nc.compile()
    return nc


def _prep_inputs(query, key_, value, Wq, bq, Wk, bk, Wv, bv, Wo, bo, Wg, bg):
    """Host-side sharding / layout prep. Returns in_maps for the 8 cores."""
    f32 = np.float32

    def bf(x):
        return np.ascontiguousarray(np.asarray(x, f32)).astype(BF16_NP)

    wq_b, wk_b, wv_b, wo_b = bf(Wq), bf(Wk), bf(Wv), bf(Wo)
    wg_b = np.ascontiguousarray(
        bf(Wg).reshape(DT, P, H).transpose(1, 0, 2))
    bq_pm = np.ascontiguousarray(np.asarray(bq, f32).reshape(DT, P).T)
    bk_pm = np.ascontiguousarray(np.asarray(bk, f32).reshape(DT, P).T)
    bv_b = np.ascontiguousarray(
        np.broadcast_to(np.asarray(bv, f32).astype(BF16_NP), (P, D)))
    bo_b = np.ascontiguousarray(
        np.broadcast_to(np.asarray(bo, f32).astype(BF16_NP), (P, D)))
    bg_c = np.ascontiguousarray(np.asarray(bg, f32).reshape(H, 1))

    qT = [np.asarray(query[b], f32).T for b in range(B)]
    kT = [np.asarray(key_[b], f32).T for b in range(B)]
    vT = [np.asarray(value[b], f32).T for b in range(B)]

    in_maps = []
    for c in range(N_CORES):
        b, r = c // R, c % R
        rows = slice(SL * r, SL * (r + 1))
        in_maps.append({
            "xqT": np.ascontiguousarray(qT[b][:, rows]).astype(BF16_NP),
            "xkT": np.ascontiguousarray(kT[b][:, rows]).astype(BF16_NP),
            "xvT": np.ascontiguousarray(vT[b][:, rows]).astype(BF16_NP),
            "wq": wq_b, "wk": wk_b, "wv": wv_b, "wo": wo_b, "wg": wg_b,
            "bq": bq_pm, "bk": bk_pm, "bvb": bv_b, "bob": bo_b, "bg": bg_c,
        })
    return in_maps


def kernel(query, key_, value, Wq, bq, Wk, bk, Wv, bv, Wo, bo, Wg, bg):
    global LAST_EXEC_TIME_NS
    if "nc" not in _CACHE:
        _CACHE["nc"] = _build()
    nc = _CACHE["nc"]

    in_maps = _prep_inputs(query, key_, value, Wq, bq, Wk, bk, Wv, bv,
                           Wo, bo, Wg, bg)
    trace = bool(os.environ.get("BASS_TRACE"))
    res = run_bass_kernel_spmd(nc, in_maps, core_ids=list(range(N_CORES)),
                               trace=trace)
    LAST_EXEC_TIME_NS = res.exec_time_ns

    out = np.empty((B, S, D), np.float32)
    for c in range(N_CORES):
        b, r = c // R, c % R
        out[b, SL * r:SL * (r + 1), :] = res.results[c]["out"]
    return out
